# revision 19
# baseline (speedup 1.0000x reference)
"""Trainium2 Bass kernel for nn_AliAttention (dual-score attention).

Math (reference):
  v/k/q = x @ W* + b*;  k_fei/q_fei = x_knowledge @ W*f + b*f   (512->512)
  q2 = (q @ Wq2 + bq2).reshape(B, H, L, DK)   # RAW reshape!
  ... likewise k2, qf2, kf2; v reshaped raw too.
  att = (q2 @ k2^T + qf2 @ kf2^T) / sqrt(2*DK); P = softmax(att)
  out = (P @ v).reshape(B, L, D_FF) @ Wo + bo

The raw reshape (B, L, 512) -> (B, H=8, L, DK=64) means head h of batch b
depends ONLY on rows 256h:256(h+1) of x[b] / x_knowledge[b]:
  Q[b,h][8l+g, d] = q2[l, 64g+d]  with q2 computed from the 256-row slice.
So the 16 (b,h) pairs are fully independent -> 2 pairs per NeuronCore, no
collectives.

Per pair, the score matrix has block structure: for feature-groups (gq, gk):
  S.T[gk][gq](lk, lq) = sum_d k2[lk,64gk+d] q2[lq,64gq+d] + (fei terms)
Both dual terms are computed in ONE K=128 matmul by stacking [k2|kf2] and
[q2|qf2] along the contraction (partition) axis.  Softmax denominators come
free from an extra ones-column appended to V in the P@V matmul; 1/d is
computed as exp(-ln(d)) on the scalar engine (DVE reciprocal is ~9 cyc/el).

Feature-groups live in SLOT order perm=[0,2,4,6,1,3,5,7] so the two
partition halves of each projection PSUM tile map to contiguous slot
ranges (single consolidated bias-add per half); the host permutes Wo's
rows to match.  Matmul operands are bf16 (fp32 PSUM accumulation);
x slices arrive pre-transposed/pre-cast from the host.
"""

import numpy as np
import ml_dtypes

B, L, D_MODEL, D_FF, H = 2, 2048, 512, 512, 8
DK = D_FF // H            # 64
ROWS = L // H             # 256 rows of x per (b,h) pair
N_CORES = 8
PAIRS_PER_CORE = (B * H) // N_CORES  # 2
SCALE = float(1.0 / np.sqrt(2.0 * DK).astype(np.float32))
PERM = [0, 2, 4, 6, 1, 3, 5, 7]  # slot s holds feature-group PERM[s]

_W_NAMES = ["Wv", "Wk", "Wq", "Wkf", "Wqf", "Wq2", "Wk2", "Wqf2", "Wkf2", "Wo"]
_BB_NAMES = ["bk", "bq", "bkf", "bqf", "bq2", "bk2", "bqf2", "bkf2"]  # bf16, big-bcast

_CACHE = {}


def _patch_tile(tile_mod, mybir):
    """Work around walrus-codegen limitations in this container:
    1. sem-waits attached to CTRL-class instructions (the Tile exit drain)
       fail codegen -> emit standalone single-wait EventSemaphores instead.
    2. instructions carrying more waits than their ISA struct allows fail
       codegen -> post-pass splits excess waits into standalone
       EventSemaphore instructions.
    """
    if getattr(tile_mod.TileContext, "_ali_patched", False):
        return
    from concourse.vector_clock import ScopedClock

    def _drain_and_barrier(self, tick_clock, wait_clock):
        assert self.sems is not None
        allocated = list(self.sems.allocated().values())
        if allocated:
            probe = self.nc.sync.wait_ge(allocated[0], 0)
            wait_clock.add_sem_waits(
                probe.ins, ScopedClock({None: tick_clock.global_clock})
            )
            waits = {}
            for w in probe.ins.sync_info.on_wait:
                if w.wait_value and w.wait_value > waits.get(w.id, 0):
                    waits[w.id] = w.wait_value
            probe.ins.sync_info = mybir.SyncInfo(on_wait=[], on_update=[])
            num2h = {h.num: h for h in allocated}
            for sid, val in waits.items():
                assert sid in num2h, f"wait on unknown sem {sid}"
                self.nc.sync.wait_ge(num2h[sid], val)
        self.nc.sync.drain()
        self.nc.all_engine_barrier()
        popped = self.nc._tile_sem_poison_stack.pop()
        assert popped is self._sem_poison
        self.nc.clear_and_free_semaphores(allocated)
        self.nc.all_engine_barrier()

    tile_mod.TileContext._drain_and_barrier = _drain_and_barrier
    tile_mod.TileContext._ali_patched = True


def _split_excess_waits(nc, mybir, default_keep=1):
    counter = 0
    for f in nc.m.functions:
        for bb in f.blocks:
            il = bb.instructions
            new_il = []
            changed = False
            for inst in il:
                si = inst.sync_info
                waits = list(si.on_wait) if si is not None else []
                opname = str(inst.opcode)
                if "Drain" in opname or "Nop" in opname:
                    keep = 0
                elif "Matmult" in opname or "Ldweights" in opname:
                    keep = 1
                else:
                    keep = default_keep
                if len(waits) > keep:
                    changed = True
                    moved = waits[keep:] if keep else waits
                    kept = waits[:keep] if keep else []
                    for w in moved:
                        counter += 1
                        ev = mybir.InstEventSemaphore(
                            name=f"I-wsplit-{counter}", ins=[], outs=[]
                        )
                        ev.engine = inst.engine
                        ev.sync_info = mybir.SyncInfo(on_wait=[w], on_update=[])
                        new_il.append(ev)
                    inst.sync_info = mybir.SyncInfo(
                        on_wait=kept, on_update=list(si.on_update)
                    )
                new_il.append(inst)
            if changed:
                bb.instructions = new_il


def _enable_ldw_opt():
    import concourse.bass_utils as bu

    if getattr(bu, "_ali_ldw_patched", False):
        return
    orig = bu.run_command

    def run_command_ldw(cmd, *a, **kw):
        cmd = [
            c
            for c in cmd
        ]
        return orig(cmd, *a, **kw)

    bu.run_command = run_command_ldw
    bu._ali_ldw_patched = True


def _build():
    import concourse.bass as bass
    import concourse.mybir as mybir
    import concourse.tile as tile

    _enable_ldw_opt()

    _patch_tile(tile, mybir)
    f32 = mybir.dt.float32
    bf16 = mybir.dt.bfloat16
    EXP = mybir.ActivationFunctionType.Exp
    LN = mybir.ActivationFunctionType.Ln

    nc = bass.Bass()
    P = PAIRS_PER_CORE
    # x slices arrive pre-transposed (feature-major) and pre-cast to bf16:
    # xsT[p, f, l] = x_slice_p[l, f]
    # all pre-arranged host-side so each DMA is one contiguous chunk per
    # partition: xsT[p][q, k, c] = x_slice_p[k*128+q-th feature, c-th row]
    xsT_ext = nc.declare_dram_parameter("xsT", [P, 128, 4, ROWS], bf16, isOutput=False)
    xksT_ext = nc.declare_dram_parameter("xksT", [P, 128, 4, ROWS], bf16, isOutput=False)
    w_ext = {
        n: nc.declare_dram_parameter(n, [128, 4, 512], bf16, isOutput=False)
        for n in _W_NAMES
    }
    bb_ext = {
        n: nc.declare_dram_parameter(n, [128, 4], f32, isOutput=False)
        for n in _BB_NAMES
    }
    bv_ext = nc.declare_dram_parameter("bv", [512], bf16, isOutput=False)
    bo_ext = nc.declare_dram_parameter("bo", [512], f32, isOutput=False)
    out_ext = nc.declare_dram_parameter("out", [P, ROWS, D_FF], f32, isOutput=True)

    with tile.TileContext(nc) as tc:
        with (
            tc.tile_pool(name="wts", bufs=1) as wts,
            tc.tile_pool(name="singles", bufs=1) as singles,
            tc.tile_pool(name="pairs", bufs=2) as pairs,
            tc.tile_pool(name="epool", bufs=3) as epool,
            tc.tile_pool(name="psS", bufs=2, space="PSUM") as psS,
            tc.tile_pool(name="psO", bufs=4, space="PSUM") as psO,
        ):
            # ---- x slices first so the first projection starts ASAP ----
            xTs, xkTs = [], []
            for p in range(P):
                xT = pairs.tile([128, 4, 256], bf16, tag="xT", name=f"xT{p}")
                nc.sync.dma_start(out=xT, in_=xsT_ext[p])
                xkT = pairs.tile([128, 4, 256], bf16, tag="xkT", name=f"xkT{p}")
                nc.sync.dma_start(out=xkT, in_=xksT_ext[p])
                xTs.append(xT)
                xkTs.append(xkT)
            # big broadcast biases: bb[p, m, c] = b[m*128+p] for all c.
            # DMA the (128, 4) per-partition form, then broadcast along the
            # free dim with a ones-multiply (DMA can't step-0 its fastest dim).
            ones256 = singles.tile([128, 256], bf16)
            nc.vector.memset(ones256, 1.0)
            bb_sb = {}
            for n in _BB_NAMES:
                col = singles.tile([128, 4], f32, name=f"bcol_{n}")
                nc.sync.dma_start(out=col, in_=bb_ext[n][:, :])
                t = singles.tile([128, 4, 256], bf16, name=f"bb_{n}")
                for m in range(4):
                    nc.vector.tensor_scalar_mul(
                        t[:, m, :], ones256, col[:, m:m + 1]
                    )
                bb_sb[n] = t
            ones64 = singles.tile([1, 64], bf16)
            nc.vector.memset(ones64, 1.0)
            ones128c = singles.tile([128, 1], bf16)
            nc.vector.memset(ones128c, 1.0)
            # ---- weights, chunked per-k, in order of first use ----
            w_sb = {}
            for n in ["Wq", "Wk", "Wqf", "Wkf", "Wv", "Wq2", "Wk2", "Wqf2",
                      "Wkf2", "Wo"]:
                t = wts.tile([128, 4, 512], bf16, name=f"w_{n}")
                if n in ("Wq", "Wk", "Wqf", "Wkf"):
                    # spread the critical-path weights across DMA queues
                    for k in range(4):
                        nc.sync.dma_start(out=t[:, k, :], in_=w_ext[n][:, k, :])
                else:
                    nc.sync.dma_start(out=t, in_=w_ext[n][:, :, :])
                w_sb[n] = t
            bv_bc = singles.tile([128, 512], bf16)
            nc.sync.dma_start(out=bv_bc, in_=bv_ext[:].partition_broadcast(128))
            bo_bc = singles.tile([128, 512], f32)
            nc.sync.dma_start(out=bo_bc, in_=bo_ext[:].partition_broadcast(128))

            for p in range(P):
                xT = xTs[p]
                xkT = xkTs[p]

                # ---- Phase B: first projections (one psum + one bias-add) ----
                firsts = {}
                for n, (srcT, wn, bn) in {
                    "q": (xT, "Wq", "bq"),
                    "k": (xT, "Wk", "bk"),
                    "qf": (xkT, "Wqf", "bqf"),
                    "kf": (xkT, "Wkf", "bkf"),
                }.items():
                    dst = pairs.tile([128, 4, 256], bf16, tag=f"first_{n}")
                    ps = psS.tile([128, 1024], f32, tag="S")
                    for m in range(4):
                        for k in range(4):
                            nc.tensor.matmul(
                                ps[:, m * 256:(m + 1) * 256],
                                w_sb[wn][:, k, m * 128:(m + 1) * 128],
                                srcT[:, k, :],
                                start=(k == 0),
                                stop=(k == 3),
                            )
                    nc.vector.tensor_add(
                        dst, ps.rearrange("p (m c) -> p m c", c=256), bb_sb[bn]
                    )
                    firsts[n] = dst

                # v (row-major) into VO = [V | 1] blocks, slot order
                vo = pairs.tile([128, 2, 8, 65], bf16, tag="vo")
                for t in range(2):
                    ps = psS.tile([128, 512], f32, tag="S")
                    for k in range(4):
                        nc.tensor.matmul(
                            ps,
                            xT[:, k, t * 128:(t + 1) * 128],
                            w_sb["Wv"][:, k, :],
                            start=(k == 0),
                            stop=(k == 3),
                        )
                    ps_v = ps.rearrange("p (g two d) -> p g two d", two=2, d=64)
                    bv_v = bv_bc.rearrange("p (g two d) -> p g two d", two=2, d=64)
                    # even feature-groups -> slots 0:4, odd -> slots 4:8
                    nc.vector.tensor_add(
                        vo[:, t, 0:4, 0:64], ps_v[:, :, 0, :], bv_v[:, :, 0, :]
                    )
                    nc.vector.tensor_add(
                        vo[:, t, 4:8, 0:64], ps_v[:, :, 1, :], bv_v[:, :, 1, :]
                    )
                    nc.vector.memset(vo[:, t, :, 64], 1.0)

                # ---- Phase C: second projections into stacked QS/KS ----
                # rows 0:64 = base terms, rows 64:128 = fei terms; psum rows
                # 0:64 are even feature-groups -> slots 0:4, rows 64:128 odd
                # -> slots 4:8.
                QS = pairs.tile([128, 8, 256], bf16, tag="QS")
                KS = pairs.tile([128, 8, 256], bf16, tag="KS")
                for srcn, wn, bn, half, dst in (
                    ("q", "Wq2", "bq2", 0, QS),
                    ("qf", "Wqf2", "bqf2", 64, QS),
                    ("k", "Wk2", "bk2", 0, KS),
                    ("kf", "Wkf2", "bkf2", 64, KS),
                ):
                    src = firsts[srcn]
                    ps = psS.tile([128, 1024], f32, tag="S")
                    for m in range(4):
                        for k in range(4):
                            nc.tensor.matmul(
                                ps[:, m * 256:(m + 1) * 256],
                                w_sb[wn][:, k, m * 128:(m + 1) * 128],
                                src[:, k, :],
                                start=(k == 0),
                                stop=(k == 3),
                            )
                    ps_m = ps.rearrange("p (m c) -> p m c", c=256)
                    nc.vector.tensor_add(
                        dst[half:half + 64, 0:4, :],
                        ps_m[0:64, :, :],
                        bb_sb[bn][0:64, :, :],
                    )
                    nc.vector.tensor_add(
                        dst[half:half + 64, 4:8, :],
                        ps_m[64:128, :, :],
                        bb_sb[bn][64:128, :, :],
                    )

                # ---- Phase D: attention (slot-indexed everywhere) ----
                # ot[j] rows 0:64 = P@V of chunk j (array col-groups 0-1),
                # row 64 = softmax denominators (ones matmul, col-group 2) —
                # the two matmuls stream the same E half concurrently in
                # disjoint column strips.
                ot = [
                    psO.tile([65, 512], f32, tag="OT", name=f"ot{j}")
                    for j in range(4)
                ]
                for gk in range(8):
                    for c2 in range(2):
                        first = gk == 0 and c2 == 0
                        last = gk == 7 and c2 == 1
                        for half in range(2):
                            s_ps = psS.tile([128, 1024], f32, tag="S")
                            for q in range(2):
                                nc.tensor.matmul(
                                    s_ps[:, q * 512:(q + 1) * 512],
                                    KS[:, gk, c2 * 128:(c2 + 1) * 128],
                                    QS[:, 4 * half + 2 * q: 4 * half + 2 * q + 2, :],
                                    start=True,
                                    stop=True,
                                )
                            e = epool.tile([128, 1024], bf16, tag="E")
                            nc.scalar.activation(e, s_ps, EXP, scale=SCALE)
                            for q in range(2):
                                nc.tensor.matmul(
                                    ot[2 * half + q],
                                    vo[:, c2, gk, :],
                                    e[:, q * 512:(q + 1) * 512],
                                    start=first,
                                    stop=last,
                                )

                # ---- Phase E: normalize rows by the ones-column sums ----
                # 1/denom = exp(-ln(denom)) on ScalarE (DVE recip is slow)
                out2dT = pairs.tile([128, 4, 256], bf16, tag="out2dT")
                for j in range(4):
                    den_b = pairs.tile([1, 512], bf16, tag="den_b")
                    nc.vector.tensor_copy(den_b, ot[j][64:65, :])
                    bc_ps = psS.tile([64, 512], f32, tag="S")
                    nc.tensor.matmul(bc_ps, ones64, den_b, start=True, stop=True)
                    ln_sb = pairs.tile([64, 512], f32, tag="ln_sb")
                    nc.scalar.activation(ln_sb, bc_ps, LN)
                    rec_sb = pairs.tile([64, 512], f32, tag="rec_sb")
                    nc.scalar.activation(rec_sb, ln_sb, EXP, scale=-1.0)
                    nc.vector.tensor_mul(
                        out2dT[0:64, j, :], ot[j][0:64, 0:256], rec_sb[:, 0:256]
                    )
                    nc.vector.tensor_mul(
                        out2dT[64:128, j, :], ot[j][0:64, 256:512], rec_sb[:, 256:512]
                    )

                # ---- Phase F: final projection + bias, DMA out ----
                y_sb = pairs.tile([128, 2, 512], f32, tag="y")
                for t in range(2):
                    ps = psS.tile([128, 512], f32, tag="S")
                    for k in range(4):
                        nc.tensor.matmul(
                            ps,
                            out2dT[:, k, t * 128:(t + 1) * 128],
                            w_sb["Wo"][:, k, :],
                            start=(k == 0),
                            stop=(k == 3),
                        )
                    nc.vector.tensor_add(y_sb[:, t, :], ps, bo_bc)
                    nc.sync.dma_start(
                        out=out_ext[p, t * 128:(t + 1) * 128, :], in_=y_sb[:, t, :]
                    )

    _split_excess_waits(nc, mybir)
    return nc


def _get_nc():
    if "nc" not in _CACHE:
        _CACHE["nc"] = _build()
    return _CACHE["nc"]


def kernel(**inputs) -> np.ndarray:
    from concourse.bass_utils import run_bass_kernel_spmd

    x = np.ascontiguousarray(inputs["x"], dtype=np.float32)
    xk = np.ascontiguousarray(inputs["x_knowledge"], dtype=np.float32)

    bf = ml_dtypes.bfloat16
    shared = {}
    for n in _W_NAMES:
        w = np.ascontiguousarray(inputs[n], dtype=np.float32)
        if n == "Wo":
            # permute rows into slot order: slot s holds group PERM[s]
            w = w.reshape(8, 64, 512)[PERM].reshape(512, 512)
        # SBUF layout: [partition p, k, n] = W[k*128+p, n]
        shared[n] = np.ascontiguousarray(
            w.reshape(4, 128, 512).transpose(1, 0, 2)
        ).astype(bf)
    for n in _BB_NAMES:
        b = np.ascontiguousarray(inputs[n], dtype=np.float32)
        shared[n] = np.ascontiguousarray(b.reshape(4, 128).T)
    shared["bo"] = np.ascontiguousarray(inputs["bo"], dtype=np.float32)
    shared["bv"] = np.ascontiguousarray(inputs["bv"], dtype=np.float32).astype(bf)

    in_maps = []
    for c in range(N_CORES):
        m = dict(shared)
        xsT = np.empty((PAIRS_PER_CORE, 128, 4, ROWS), bf)
        xksT = np.empty((PAIRS_PER_CORE, 128, 4, ROWS), bf)
        for i in range(PAIRS_PER_CORE):
            pair = c * PAIRS_PER_CORE + i
            b, h = divmod(pair, H)
            xs = x[b, h * ROWS:(h + 1) * ROWS, :].T  # (512 feat, 256 rows)
            xsT[i] = xs.reshape(4, 128, ROWS).transpose(1, 0, 2).astype(bf)
            xks = xk[b, h * ROWS:(h + 1) * ROWS, :].T
            xksT[i] = xks.reshape(4, 128, ROWS).transpose(1, 0, 2).astype(bf)
        m["xsT"] = xsT
        m["xksT"] = xksT
        in_maps.append(m)

    nc = _get_nc()
    res = run_bass_kernel_spmd(nc, in_maps, core_ids=list(range(N_CORES)))

    out = np.empty((B, L, D_FF), np.float32)
    for c in range(N_CORES):
        o = res.results[c]["out"]
        for i in range(PAIRS_PER_CORE):
            pair = c * PAIRS_PER_CORE + i
            b, h = divmod(pair, H)
            out[b, h * ROWS:(h + 1) * ROWS, :] = o[i]
    return out


# revision 20
# speedup vs baseline: 1.1380x; 1.1380x over previous
"""Trainium2 Bass kernel for nn_AliAttention (dual-score attention).

Math (reference):
  v/k/q = x @ W* + b*;  k_fei/q_fei = x_knowledge @ W*f + b*f   (512->512)
  q2 = (q @ Wq2 + bq2).reshape(B, H, L, DK)   # RAW reshape!
  ... likewise k2, qf2, kf2; v reshaped raw too.
  att = (q2 @ k2^T + qf2 @ kf2^T) / sqrt(2*DK); P = softmax(att)
  out = (P @ v).reshape(B, L, D_FF) @ Wo + bo

The raw reshape (B, L, 512) -> (B, H=8, L, DK=64) means head h of batch b
depends ONLY on rows 256h:256(h+1) of x[b] / x_knowledge[b]:
  Q[b,h][8l+g, d] = q2[l, 64g+d]  with q2 computed from the 256-row slice.
So the 16 (b,h) pairs are fully independent -> 2 pairs per NeuronCore, no
collectives.

Per pair, the score matrix has block structure: for feature-groups (gq, gk):
  S.T[gk][gq](lk, lq) = sum_d k2[lk,64gk+d] q2[lq,64gq+d] + (fei terms)
Both dual terms are computed in ONE K=128 matmul by stacking [k2|kf2] and
[q2|qf2] along the contraction (partition) axis.  Softmax denominators come
free from an extra ones-column appended to V in the P@V matmul; 1/d is
computed as exp(-ln(d)) on the scalar engine (DVE reciprocal is ~9 cyc/el).

Feature-groups live in SLOT order perm=[0,2,4,6,1,3,5,7] so the two
partition halves of each projection PSUM tile map to contiguous slot
ranges (single consolidated bias-add per half); the host permutes Wo's
rows to match.  Matmul operands are bf16 (fp32 PSUM accumulation);
x slices arrive pre-transposed/pre-cast from the host.
"""

import numpy as np
import ml_dtypes

B, L, D_MODEL, D_FF, H = 2, 2048, 512, 512, 8
DK = D_FF // H            # 64
ROWS = L // H             # 256 rows of x per (b,h) pair
N_CORES = 8
PAIRS_PER_CORE = (B * H) // N_CORES  # 2
SCALE = float(1.0 / np.sqrt(2.0 * DK).astype(np.float32))
PERM = [0, 2, 4, 6, 1, 3, 5, 7]  # slot s holds feature-group PERM[s]

_W_NAMES = ["Wv", "Wqe", "Wke", "Wqfe", "Wkfe", "Wo"]
_BB_NAMES = ["bqe", "bke", "bqfe", "bkfe"]  # effective biases, big-bcast

_CACHE = {}


def _patch_tile(tile_mod, mybir):
    """Work around walrus-codegen limitations in this container:
    1. sem-waits attached to CTRL-class instructions (the Tile exit drain)
       fail codegen -> emit standalone single-wait EventSemaphores instead.
    2. instructions carrying more waits than their ISA struct allows fail
       codegen -> post-pass splits excess waits into standalone
       EventSemaphore instructions.
    """
    if getattr(tile_mod.TileContext, "_ali_patched", False):
        return
    from concourse.vector_clock import ScopedClock

    def _drain_and_barrier(self, tick_clock, wait_clock):
        assert self.sems is not None
        allocated = list(self.sems.allocated().values())
        if allocated:
            probe = self.nc.sync.wait_ge(allocated[0], 0)
            wait_clock.add_sem_waits(
                probe.ins, ScopedClock({None: tick_clock.global_clock})
            )
            waits = {}
            for w in probe.ins.sync_info.on_wait:
                if w.wait_value and w.wait_value > waits.get(w.id, 0):
                    waits[w.id] = w.wait_value
            probe.ins.sync_info = mybir.SyncInfo(on_wait=[], on_update=[])
            num2h = {h.num: h for h in allocated}
            for sid, val in waits.items():
                assert sid in num2h, f"wait on unknown sem {sid}"
                self.nc.sync.wait_ge(num2h[sid], val)
        self.nc.sync.drain()
        self.nc.all_engine_barrier()
        popped = self.nc._tile_sem_poison_stack.pop()
        assert popped is self._sem_poison
        self.nc.clear_and_free_semaphores(allocated)
        self.nc.all_engine_barrier()

    tile_mod.TileContext._drain_and_barrier = _drain_and_barrier
    tile_mod.TileContext._ali_patched = True


def _split_excess_waits(nc, mybir, default_keep=1):
    counter = 0
    for f in nc.m.functions:
        for bb in f.blocks:
            il = bb.instructions
            new_il = []
            changed = False
            for inst in il:
                si = inst.sync_info
                waits = list(si.on_wait) if si is not None else []
                opname = str(inst.opcode)
                if "Drain" in opname or "Nop" in opname:
                    keep = 0
                elif "Matmult" in opname or "Ldweights" in opname:
                    keep = 1
                else:
                    keep = default_keep
                if len(waits) > keep:
                    changed = True
                    moved = waits[keep:] if keep else waits
                    kept = waits[:keep] if keep else []
                    for w in moved:
                        counter += 1
                        ev = mybir.InstEventSemaphore(
                            name=f"I-wsplit-{counter}", ins=[], outs=[]
                        )
                        ev.engine = inst.engine
                        ev.sync_info = mybir.SyncInfo(on_wait=[w], on_update=[])
                        new_il.append(ev)
                    inst.sync_info = mybir.SyncInfo(
                        on_wait=kept, on_update=list(si.on_update)
                    )
                new_il.append(inst)
            if changed:
                bb.instructions = new_il


def _enable_ldw_opt():
    import concourse.bass_utils as bu

    if getattr(bu, "_ali_ldw_patched", False):
        return
    orig = bu.run_command

    def run_command_ldw(cmd, *a, **kw):
        cmd = [
            c
            for c in cmd
        ]
        return orig(cmd, *a, **kw)

    bu.run_command = run_command_ldw
    bu._ali_ldw_patched = True


def _build():
    import concourse.bass as bass
    import concourse.mybir as mybir
    import concourse.tile as tile

    _enable_ldw_opt()

    _patch_tile(tile, mybir)
    f32 = mybir.dt.float32
    bf16 = mybir.dt.bfloat16
    EXP = mybir.ActivationFunctionType.Exp
    LN = mybir.ActivationFunctionType.Ln

    nc = bass.Bass()
    P = PAIRS_PER_CORE
    # x slices arrive pre-transposed (feature-major) and pre-cast to bf16:
    # xsT[p, f, l] = x_slice_p[l, f]
    # all pre-arranged host-side so each DMA is one contiguous chunk per
    # partition: xsT[p][q, k, c] = x_slice_p[k*128+q-th feature, c-th row]
    xsT_ext = nc.declare_dram_parameter("xsT", [P, 128, 4, ROWS], bf16, isOutput=False)
    xksT_ext = nc.declare_dram_parameter("xksT", [P, 128, 4, ROWS], bf16, isOutput=False)
    w_ext = {
        n: nc.declare_dram_parameter(n, [128, 4, 512], bf16, isOutput=False)
        for n in _W_NAMES
    }
    bb_ext = {
        n: nc.declare_dram_parameter(n, [128, 4], f32, isOutput=False)
        for n in _BB_NAMES
    }
    bv_ext = nc.declare_dram_parameter("bv", [512], bf16, isOutput=False)
    bo_ext = nc.declare_dram_parameter("bo", [512], f32, isOutput=False)
    out_ext = nc.declare_dram_parameter("out", [P, ROWS, D_FF], f32, isOutput=True)

    with tile.TileContext(nc) as tc:
        with (
            tc.tile_pool(name="wts", bufs=1) as wts,
            tc.tile_pool(name="singles", bufs=1) as singles,
            tc.tile_pool(name="pairs", bufs=2) as pairs,
            tc.tile_pool(name="epool", bufs=3) as epool,
            tc.tile_pool(name="psS", bufs=2, space="PSUM") as psS,
            tc.tile_pool(name="psO", bufs=4, space="PSUM") as psO,
        ):
            # ---- x slices first so the first projection starts ASAP ----
            xTs, xkTs = [], []
            for p in range(P):
                xT = pairs.tile([128, 4, 256], bf16, tag="xT", name=f"xT{p}")
                nc.sync.dma_start(out=xT, in_=xsT_ext[p])
                xkT = pairs.tile([128, 4, 256], bf16, tag="xkT", name=f"xkT{p}")
                nc.sync.dma_start(out=xkT, in_=xksT_ext[p])
                xTs.append(xT)
                xkTs.append(xkT)
            # big broadcast biases: bb[p, m, c] = b[m*128+p] for all c.
            # DMA the (128, 4) per-partition form, then broadcast along the
            # free dim with a ones-multiply (DMA can't step-0 its fastest dim).
            ones256 = singles.tile([128, 256], bf16)
            nc.vector.memset(ones256, 1.0)
            bb_sb = {}
            for n in _BB_NAMES:
                col = singles.tile([128, 4], f32, name=f"bcol_{n}")
                nc.sync.dma_start(out=col, in_=bb_ext[n][:, :])
                t = singles.tile([128, 4, 256], bf16, name=f"bb_{n}")
                for m in range(4):
                    nc.vector.tensor_scalar_mul(
                        t[:, m, :], ones256, col[:, m:m + 1]
                    )
                bb_sb[n] = t
            ones64 = singles.tile([1, 64], bf16)
            nc.vector.memset(ones64, 1.0)
            ones128c = singles.tile([128, 1], bf16)
            nc.vector.memset(ones128c, 1.0)
            # ---- weights, chunked per-k, in order of first use ----
            w_sb = {}
            for n in ["Wqe", "Wke", "Wqfe", "Wkfe", "Wv", "Wo"]:
                t = wts.tile([128, 4, 512], bf16, name=f"w_{n}")
                nc.sync.dma_start(out=t, in_=w_ext[n][:, :, :])
                w_sb[n] = t
            bv_bc = singles.tile([128, 512], bf16)
            nc.sync.dma_start(out=bv_bc, in_=bv_ext[:].partition_broadcast(128))
            bo_bc = singles.tile([128, 512], f32)
            nc.sync.dma_start(out=bo_bc, in_=bo_ext[:].partition_broadcast(128))

            for p in range(P):
                xT = xTs[p]
                xkT = xkTs[p]

                # v (row-major) into VO = [V | 1] blocks, slot order
                vo = pairs.tile([128, 2, 8, 65], bf16, tag="vo")
                for t in range(2):
                    ps = psS.tile([128, 512], f32, tag="S")
                    for k in range(4):
                        nc.tensor.matmul(
                            ps,
                            xT[:, k, t * 128:(t + 1) * 128],
                            w_sb["Wv"][:, k, :],
                            start=(k == 0),
                            stop=(k == 3),
                        )
                    ps_v = ps.rearrange("p (g two d) -> p g two d", two=2, d=64)
                    bv_v = bv_bc.rearrange("p (g two d) -> p g two d", two=2, d=64)
                    # even feature-groups -> slots 0:4, odd -> slots 4:8
                    nc.vector.tensor_add(
                        vo[:, t, 0:4, 0:64], ps_v[:, :, 0, :], bv_v[:, :, 0, :]
                    )
                    nc.vector.tensor_add(
                        vo[:, t, 4:8, 0:64], ps_v[:, :, 1, :], bv_v[:, :, 1, :]
                    )
                    nc.vector.memset(vo[:, t, :, 64], 1.0)

                # ---- Phase B/C: fused projections into stacked QS/KS ----
                # (no nonlinearity between the two reference projections, so
                # W_eff = W1 @ W2 and b_eff = b1 @ W2 + b2 are host-fused)
                # rows 0:64 = base terms, rows 64:128 = fei terms; psum rows
                # 0:64 are even feature-groups -> slots 0:4, rows 64:128 odd
                # -> slots 4:8.
                QS = pairs.tile([128, 8, 256], bf16, tag="QS")
                KS = pairs.tile([128, 8, 256], bf16, tag="KS")
                for src, wn, bn, half, dst in (
                    (xT, "Wqe", "bqe", 0, QS),
                    (xkT, "Wqfe", "bqfe", 64, QS),
                    (xT, "Wke", "bke", 0, KS),
                    (xkT, "Wkfe", "bkfe", 64, KS),
                ):
                    ps = psS.tile([128, 1024], f32, tag="S")
                    for m in range(4):
                        for k in range(4):
                            nc.tensor.matmul(
                                ps[:, m * 256:(m + 1) * 256],
                                w_sb[wn][:, k, m * 128:(m + 1) * 128],
                                src[:, k, :],
                                start=(k == 0),
                                stop=(k == 3),
                            )
                    ps_m = ps.rearrange("p (m c) -> p m c", c=256)
                    nc.vector.tensor_add(
                        dst[half:half + 64, 0:4, :],
                        ps_m[0:64, :, :],
                        bb_sb[bn][0:64, :, :],
                    )
                    nc.vector.tensor_add(
                        dst[half:half + 64, 4:8, :],
                        ps_m[64:128, :, :],
                        bb_sb[bn][64:128, :, :],
                    )

                # ---- Phase D: attention (slot-indexed everywhere) ----
                # ot[j] rows 0:64 = P@V of chunk j (array col-groups 0-1),
                # row 64 = softmax denominators (ones matmul, col-group 2) —
                # the two matmuls stream the same E half concurrently in
                # disjoint column strips.
                ot = [
                    psO.tile([65, 512], f32, tag="OT", name=f"ot{j}")
                    for j in range(4)
                ]
                for gk in range(8):
                    for c2 in range(2):
                        first = gk == 0 and c2 == 0
                        last = gk == 7 and c2 == 1
                        for half in range(2):
                            s_ps = psS.tile([128, 1024], f32, tag="S")
                            for q in range(2):
                                nc.tensor.matmul(
                                    s_ps[:, q * 512:(q + 1) * 512],
                                    KS[:, gk, c2 * 128:(c2 + 1) * 128],
                                    QS[:, 4 * half + 2 * q: 4 * half + 2 * q + 2, :],
                                    start=True,
                                    stop=True,
                                )
                            e = epool.tile([128, 1024], bf16, tag="E")
                            nc.scalar.activation(e, s_ps, EXP, scale=SCALE)
                            for q in range(2):
                                nc.tensor.matmul(
                                    ot[2 * half + q],
                                    vo[:, c2, gk, :],
                                    e[:, q * 512:(q + 1) * 512],
                                    start=first,
                                    stop=last,
                                )

                # ---- Phase E: normalize rows by the ones-column sums ----
                # 1/denom = exp(-ln(denom)) on ScalarE (DVE recip is slow)
                out2dT = pairs.tile([128, 4, 256], bf16, tag="out2dT")
                for j in range(4):
                    den_b = pairs.tile([1, 512], bf16, tag="den_b")
                    nc.vector.tensor_copy(den_b, ot[j][64:65, :])
                    bc_ps = psS.tile([64, 512], f32, tag="S")
                    nc.tensor.matmul(bc_ps, ones64, den_b, start=True, stop=True)
                    ln_sb = pairs.tile([64, 512], f32, tag="ln_sb")
                    nc.scalar.activation(ln_sb, bc_ps, LN)
                    rec_sb = pairs.tile([64, 512], f32, tag="rec_sb")
                    nc.scalar.activation(rec_sb, ln_sb, EXP, scale=-1.0)
                    nc.vector.tensor_mul(
                        out2dT[0:64, j, :], ot[j][0:64, 0:256], rec_sb[:, 0:256]
                    )
                    nc.vector.tensor_mul(
                        out2dT[64:128, j, :], ot[j][0:64, 256:512], rec_sb[:, 256:512]
                    )

                # ---- Phase F: final projection + bias, DMA out ----
                y_sb = pairs.tile([128, 2, 512], f32, tag="y")
                for t in range(2):
                    ps = psS.tile([128, 512], f32, tag="S")
                    for k in range(4):
                        nc.tensor.matmul(
                            ps,
                            out2dT[:, k, t * 128:(t + 1) * 128],
                            w_sb["Wo"][:, k, :],
                            start=(k == 0),
                            stop=(k == 3),
                        )
                    nc.vector.tensor_add(y_sb[:, t, :], ps, bo_bc)
                    nc.sync.dma_start(
                        out=out_ext[p, t * 128:(t + 1) * 128, :], in_=y_sb[:, t, :]
                    )

    _split_excess_waits(nc, mybir)
    return nc


def _get_nc():
    if "nc" not in _CACHE:
        _CACHE["nc"] = _build()
    return _CACHE["nc"]


def kernel(**inputs) -> np.ndarray:
    from concourse.bass_utils import run_bass_kernel_spmd

    x = np.ascontiguousarray(inputs["x"], dtype=np.float32)
    xk = np.ascontiguousarray(inputs["x_knowledge"], dtype=np.float32)

    bf = ml_dtypes.bfloat16
    f64 = np.float64
    eff_w = {}
    eff_b = {}
    for en, w1n, b1n, w2n, b2n in (
        ("Wqe", "Wq", "bq", "Wq2", "bq2"),
        ("Wke", "Wk", "bk", "Wk2", "bk2"),
        ("Wqfe", "Wqf", "bqf", "Wqf2", "bqf2"),
        ("Wkfe", "Wkf", "bkf", "Wkf2", "bkf2"),
    ):
        w1 = np.asarray(inputs[w1n], f64)
        w2 = np.asarray(inputs[w2n], f64)
        eff_w[en] = (w1 @ w2).astype(np.float32)
        eff_b["b" + en[1:]] = (
            np.asarray(inputs[b1n], f64) @ w2 + np.asarray(inputs[b2n], f64)
        ).astype(np.float32)
    eff_w["Wv"] = np.ascontiguousarray(inputs["Wv"], dtype=np.float32)
    eff_w["Wo"] = np.ascontiguousarray(inputs["Wo"], dtype=np.float32)

    shared = {}
    for n in _W_NAMES:
        w = eff_w[n]
        if n == "Wo":
            # permute rows into slot order: slot s holds group PERM[s]
            w = w.reshape(8, 64, 512)[PERM].reshape(512, 512)
        # SBUF layout: [partition p, k, n] = W[k*128+p, n]
        shared[n] = np.ascontiguousarray(
            w.reshape(4, 128, 512).transpose(1, 0, 2)
        ).astype(bf)
    for n in _BB_NAMES:
        b = eff_b[n]
        shared[n] = np.ascontiguousarray(b.reshape(4, 128).T)
    shared["bo"] = np.ascontiguousarray(inputs["bo"], dtype=np.float32)
    shared["bv"] = np.ascontiguousarray(inputs["bv"], dtype=np.float32).astype(bf)

    in_maps = []
    for c in range(N_CORES):
        m = dict(shared)
        xsT = np.empty((PAIRS_PER_CORE, 128, 4, ROWS), bf)
        xksT = np.empty((PAIRS_PER_CORE, 128, 4, ROWS), bf)
        for i in range(PAIRS_PER_CORE):
            pair = c * PAIRS_PER_CORE + i
            b, h = divmod(pair, H)
            xs = x[b, h * ROWS:(h + 1) * ROWS, :].T  # (512 feat, 256 rows)
            xsT[i] = xs.reshape(4, 128, ROWS).transpose(1, 0, 2).astype(bf)
            xks = xk[b, h * ROWS:(h + 1) * ROWS, :].T
            xksT[i] = xks.reshape(4, 128, ROWS).transpose(1, 0, 2).astype(bf)
        m["xsT"] = xsT
        m["xksT"] = xksT
        in_maps.append(m)

    nc = _get_nc()
    res = run_bass_kernel_spmd(nc, in_maps, core_ids=list(range(N_CORES)))

    out = np.empty((B, L, D_FF), np.float32)
    for c in range(N_CORES):
        o = res.results[c]["out"]
        for i in range(PAIRS_PER_CORE):
            pair = c * PAIRS_PER_CORE + i
            b, h = divmod(pair, H)
            out[b, h * ROWS:(h + 1) * ROWS, :] = o[i]
    return out
